# revision 1
# baseline (speedup 1.0000x reference)
"""Trainium2 Bass kernel for DiagonalVariational sampling.

z[n, i] = m[i] + std_normal[n, i] * (diag_L[i] + JITTER)

Sharding: std_normal split along n_sample across 8 cores (data parallel);
m and diag_L replicated. Pure elementwise -> memory-bound. Each core
streams its 32MB shard in + 32MB out with contiguous 2MB DMAs; diag_L/m
are replicated across the 128 SBUF partitions so the DVE can apply them
as [128, C] tensor operands against row-major sample tiles.
"""

import numpy as np

import concourse.bacc as bacc
import concourse.mybir as mybir
import concourse.tile as tile
from concourse.bass_utils import run_bass_kernel_spmd

D = 16384
N_SAMPLE = 4096
N_CORES = 8
ROWS = N_SAMPLE // N_CORES  # 512 sample rows per core
P = 128                     # SBUF partitions
RB = ROWS // P              # 4 row blocks per core
CCH = 4096                  # column chunk (free dim per tile)
NCH = D // CCH              # 4 column chunks
JITTER = 1e-06
DT = mybir.dt.float32

_CACHE: dict = {}


def _build_nc(repeats=1, variant="pe"):
    nc = bacc.Bacc(
        "TRN2", target_bir_lowering=False, debug=False, num_devices=N_CORES
    )
    m = nc.dram_tensor("m", [D], DT, kind="ExternalInput")
    dl = nc.dram_tensor("diag_L", [D], DT, kind="ExternalInput")
    x = nc.dram_tensor("x", [ROWS, D], DT, kind="ExternalInput")
    z = nc.dram_tensor("z", [ROWS, D], DT, kind="ExternalOutput")

    MMN = 512  # matmul free dim / one PSUM bank of f32
    cch = 8192 if variant == "big8k" else CCH
    ncch = D // cch
    xbufs = 2 if variant == "big8k" else 3
    rbufs = 1 if variant == "big8k" else 2

    with tile.TileContext(nc) as tc:
        with (
            tc.tile_pool(name="const", bufs=1) as cpool,
            tc.tile_pool(name="xt", bufs=xbufs) as xpool,
            tc.tile_pool(name="psum", bufs=4, space="PSUM") as ppool,
            tc.tile_pool(name="rows", bufs=rbufs) as rpool,
        ):
            scale_b = cpool.tile([P, D], DT)  # diag_L replicated on partitions
            m_b = cpool.tile([P, D], DT)      # m replicated on partitions

            # On-chip broadcast: ones[1,128].T @ row[1,N] -> PSUM[128,N],
            # then copy PSUM->SBUF. Avoids the 128x read-amplified HBM
            # broadcast DMA (16MB extra HBM traffic). Row vectors are
            # staged through small [1, ROWCH] chunks to bound SBUF usage.
            ROWCH = 2048
            ones = cpool.tile([1, P], DT)
            nc.vector.memset(ones[:], 1.0)

            def bcast_vec(src, dst, rc, copy_engine):
                rs_ = slice(rc * ROWCH, (rc + 1) * ROWCH)
                row = rpool.tile([1, ROWCH], DT, tag="rows")
                nc.sync.dma_start(
                    out=row[:], in_=src[rs_].rearrange("(a f) -> a f", a=1)
                )
                for j in range(ROWCH // MMN):
                    col = slice(rc * ROWCH + j * MMN, rc * ROWCH + (j + 1) * MMN)
                    ps = ppool.tile([P, MMN], DT)
                    nc.tensor.matmul(
                        ps[:],
                        ones[:],
                        row[:, j * MMN : (j + 1) * MMN],
                        start=True,
                        stop=True,
                    )
                    copy_engine(dst[:, col], ps[:])

            def emit_broadcast(c):
                if variant == "hbm":
                    cs = slice(c * cch, (c + 1) * cch)
                    nc.scalar.dma_start(
                        out=scale_b[:, cs],
                        in_=dl[cs]
                        .rearrange("(a f) -> a f", a=1)
                        .broadcast_to([P, cch]),
                    )
                    nc.scalar.dma_start(
                        out=m_b[:, cs],
                        in_=m[cs]
                        .rearrange("(a f) -> a f", a=1)
                        .broadcast_to([P, cch]),
                    )
                    return
                for h in range(cch // ROWCH):
                    rc = c * (cch // ROWCH) + h
                    bcast_vec(dl, scale_b, rc, nc.vector.tensor_copy)  # DVE
                    bcast_vec(m, m_b, rc, nc.scalar.copy)  # ACT

            do_stt = variant not in ("dmaonly", "ronly", "wonly")
            do_add = variant not in ("dmaonly", "dve1", "ronly", "wonly")
            for _r in range(repeats):
                for c in range(ncch):
                    if _r == 0:
                        emit_broadcast(c)
                    cs = slice(c * cch, (c + 1) * cch)
                    for t in range(RB):
                        rs = slice(t * P, (t + 1) * P)
                        i_tile = c * RB + t
                        if variant == "wonly":
                            eng = nc.sync if t % 2 else nc.scalar
                            eng.dma_start(out=z[rs, cs], in_=scale_b[:, cs])
                            continue
                        xt = xpool.tile([P, cch], DT)
                        if variant in ("split", "splitg", "ronly2"):
                            ld_eng = nc.sync if i_tile % 2 else nc.scalar
                            st_eng = nc.scalar if i_tile % 2 else nc.sync
                        else:
                            ld_eng, st_eng = nc.sync, nc.scalar
                        ld_eng.dma_start(out=xt[:], in_=x[rs, cs])
                        if variant in ("ronly", "ronly2"):
                            # tiny probe store keeps the load live (no DCE)
                            nc.scalar.dma_start(
                                out=z[rs, c * cch : c * cch + 8],
                                in_=xt[:, :8],
                            )
                            continue
                        if do_stt:
                            # xt = (diag_L + JITTER) * xt
                            nc.vector.scalar_tensor_tensor(
                                out=xt[:],
                                in0=scale_b[:, cs],
                                scalar=JITTER,
                                in1=xt[:],
                                op0=mybir.AluOpType.add,
                                op1=mybir.AluOpType.mult,
                            )
                        if do_add:
                            # xt += m
                            add_eng = (
                                nc.gpsimd
                                if variant in ("gpadd", "splitg")
                                else nc.vector
                            )
                            add_eng.tensor_add(xt[:], xt[:], m_b[:, cs])
                        st_eng.dma_start(out=z[rs, cs], in_=xt[:])

    nc.compile()
    return nc


def get_nc(repeats=1, variant="pe"):
    key = (repeats, variant)
    if key not in _CACHE:
        _CACHE[key] = _build_nc(repeats, variant)
    return _CACHE[key]


def run_spmd(m, diag_L, std_normal, trace=False, repeats=1):
    """Run the SPMD kernel; returns (z_full, BassKernelResults)."""
    nc = get_nc(repeats)
    m = np.ascontiguousarray(m, dtype=np.float32)
    diag_L = np.ascontiguousarray(diag_L, dtype=np.float32)
    std_normal = np.ascontiguousarray(std_normal, dtype=np.float32)
    in_maps = [
        {
            "m": m,
            "diag_L": diag_L,
            "x": std_normal[i * ROWS : (i + 1) * ROWS],
        }
        for i in range(N_CORES)
    ]
    res = run_bass_kernel_spmd(nc, in_maps, list(range(N_CORES)), trace=trace)
    z = np.concatenate([res.results[i]["z"] for i in range(N_CORES)], axis=0)
    return z, res


def kernel(m, diag_L, std_normal):
    z, _ = run_spmd(m, diag_L, std_normal)
    return z

